# revision 24
# baseline (speedup 1.0000x reference)
"""Causal self-attention (4, 2048, 1024), 16 heads, on 8 trn2 NeuronCores.

Sharding: batch (4) x head-group (2 groups of 8 heads) -> 8 cores.
Each core computes, for its batch b and its 8 heads:
  qkv projection -> causal attention -> partial output projection
  partial_out = Y_heads @ w_proj[rows of those heads]
Host sums the two head-group partials per batch. No collectives.

v2 changes vs v1 (296us baseline):
- Host-side tensors pre-arranged in exact SBUF layouts so every load is a
  few large-descriptor DMAs; chunk-0 x arrives in four 128-t slices so
  projections start ~3us into the kernel instead of ~18us.
- Projection matmuls: chunk-0 single-shot accumulates per 256-t half (x
  arrives in slices); all other chunks use one N=512 group per (m, c) so
  each weight tile is loaded once.
- Softmax denominators: the [V|ones] / [ones|V] packing puts 64 copies of
  each head's denominator row in the PV accumulator; two K=64 fp16
  selection matmuls on disjoint row/col groups (concurrent in the array,
  like the QK pair) realign [dens_A; dens_B] onto the same partitions as
  [YT_A; YT_B], replacing the per-group SBUF->SBUF DMA broadcasts that sat
  on the normalize critical path.  One reciprocal per group.
- Output in fp16, staged per chunk in SBUF and stored with one DMA per
  chunk issued from the Pool (gpsimd) queue so input prefetches on the SP
  queue are never stuck behind stores.  Host upcasts and sums partials.
- vv ones-halves initialized by Pool-engine memsets (off the DVE).
- PSUM pools: pp (proj + out-proj + den) 2 banks, st 2x2 banks, yt 2 banks.
"""

import numpy as np

import concourse.bass as bass
import concourse.mybir as mybir
import concourse.tile as tile
from concourse import bacc

F32 = mybir.dt.float32
F32R = mybir.dt.float32r
FP16 = mybir.dt.float16

T = 2048   # sequence length
C = 1024   # embed dim
NP = 4     # head pairs per core (8 heads)
NKT = 16   # k-tiles of 128
EXPF = mybir.ActivationFunctionType.Exp


def r(ap):
    return ap.bitcast(F32R)


def build_nc(repeat=1):
    nc = bacc.Bacc(trn_type="TRN2", target_bir_lowering=False, debug=False,
                   num_devices=8)
    # x, transposed+tiled: [p, tq(4), ts(4), c(8), t(128)]
    xT4 = nc.dram_tensor("xT4", [128, 4, 4, 8, 128], FP16,
                         kind="ExternalInput").ap()
    # Q/K proj weights: [p, m(8: q pairs 0-3, k pairs 4-7), c(8), n(128)]
    wqkvT = nc.dram_tensor("wqkvT", [128, 8, 8, 128], FP16,
                           kind="ExternalInput").ap()
    # V proj weights: [p, c(8), n(512)]
    wvT = nc.dram_tensor("wvT", [128, 8, 512], FP16,
                         kind="ExternalInput").ap()
    # out proj weights: [p, g(4), n(1024)]
    wpT = nc.dram_tensor("wpT", [128, 4, 1024], FP16,
                         kind="ExternalInput").ap()
    # mask2[k, i, q] = 1.0 if q >= k else 0 (same for i=0,1)
    mask2 = nc.dram_tensor("mask2", [128, 2, 128], FP16,
                           kind="ExternalInput").ap()
    # selection matrices for denominator realignment:
    # sel[:, 0]: out[m in 0:64] = rhs[64]   (dens_A from partitions 64:128)
    # sel[:, 1]: out[m in 64:128] = rhs[0]  (dens_B from partitions 0:64)
    sel = nc.dram_tensor("sel", [128, 2, 128], FP16,
                         kind="ExternalInput").ap()
    out = nc.dram_tensor("out", [T, C], FP16, kind="ExternalOutput").ap()

    with tile.TileContext(nc) as tc:
        build_body(tc, xT4, wqkvT, wvT, wpT, mask2, sel, out, repeat=repeat)
    nc.compile()
    return nc


def build_body(tc, xT4, wqkvT, wvT, wpT, mask2, sel, out, repeat=1):
    nc = tc.nc
    import contextlib
    ctx = contextlib.ExitStack()
    with ctx:
        persist = ctx.enter_context(tc.tile_pool(name="persist", bufs=1))
        xtp = ctx.enter_context(tc.tile_pool(name="xt_p", bufs=2))
        qslp = ctx.enter_context(tc.tile_pool(name="qsl_p", bufs=2))
        yslp = ctx.enter_context(tc.tile_pool(name="ysl_p", bufs=2))
        ep = ctx.enter_context(tc.tile_pool(name="e_p", bufs=12))
        rpp = ctx.enter_context(tc.tile_pool(name="rep_p", bufs=2))
        osp = ctx.enter_context(tc.tile_pool(name="osb_p", bufs=2))
        pp = ctx.enter_context(tc.tile_pool(name="pp_ps", bufs=2,
                                            space="PSUM"))
        stp = ctx.enter_context(tc.tile_pool(name="st_ps", bufs=2,
                                             space="PSUM"))
        ytp = ctx.enter_context(tc.tile_pool(name="yt_ps", bufs=2,
                                             space="PSUM"))

        kt_t = persist.tile([128, NP, T], FP16)      # KT pairs (d=128, t)
        # V+ones: per (ktile, head) block of 128 cols:
        # even heads [V_h | ones], odd heads [ones | V_h]
        vv_t = persist.tile([128, NKT, 8, 128], FP16)
        mask_t = persist.tile([128, 2, 128], FP16)
        sel_t = persist.tile([128, 2, 128], FP16)
        wv_t = persist.tile([128, 8, 512], FP16)     # V proj weights
        w_all = persist.tile([128, 8, 8, 128], FP16)  # QT/KT proj weights
        wp_t = persist.tile([128, NP, C], FP16)      # out proj weights

        # ---- one-time loads, interleaved with chunk-0 x slices on the SP
        # queue so the first projection matmuls start as early as possible.
        def w_dma(m0, m1):
            nc.sync.dma_start(out=w_all[:, m0:m1, :, :],
                              in_=wqkvT[:, m0:m1, :, :])

        # vv ones-halves (never overwritten by V copies) on the Pool engine,
        # split so the first key-tiles are ready early.
        vv5 = vv_t[:, :, :, :].rearrange("p l (hp par) d -> p l hp par d",
                                         par=2)
        for lh in range(2):
            sl = slice(8 * lh, 8 * lh + 8)
            nc.gpsimd.memset(vv5[:, sl, :, 0, 64:128], 1.0)
            nc.gpsimd.memset(vv5[:, sl, :, 1, 0:64], 1.0)

        CHUNKS = [(0, 512), (512, 512), (1024, 512), (1536, 512)]
        first = [True]
        if repeat > 1:
            # steady-state (slope) build: persist loads once, outside the
            # hardware loop, so iterations carry no reload overhead
            first[0] = False
            w_dma(0, 8)
            nc.sync.dma_start(out=wv_t[:], in_=wvT[:])
            nc.sync.dma_start(out=mask_t[:], in_=mask2[:])
            nc.sync.dma_start(out=sel_t[:], in_=sel[:])
            nc.sync.dma_start(out=wp_t[:], in_=wpT[:])

        def emit_out_proj(cq0, ysl):
            # output projection + one fp16 store per chunk (Pool queue)
            osb = osp.tile([128, 4, 1024], FP16, tag="osb")
            for tt2 in range(4):
                for ec in range(2):
                    ps = pp.tile([128, 512], F32, tag="ps")
                    for g in range(NP):
                        nc.tensor.matmul(
                            ps[:],
                            ysl[:, g, 128 * tt2:128 * (tt2 + 1)],
                            wp_t[:, g, 512 * ec:512 * (ec + 1)],
                            start=(g == 0), stop=(g == 3))
                    nc.vector.tensor_copy(
                        osb[:, tt2, 512 * ec:512 * (ec + 1)], ps[:])
            nc.gpsimd.dma_start(
                out=out[cq0:cq0 + 512, :].rearrange(
                    "(tt p) n -> p tt n", p=128),
                in_=osb[:])

        def emit_out_proj_half(cq0, ysl, gs, osb):
            # partial final-chunk projection over head-pair groups gs; the
            # second pass adds onto the first (adds split DVE/Pool) and
            # stores per 128-row piece
            second = gs[0] != 0
            for tt2 in range(4):
                for ec in range(2):
                    ps = pp.tile([128, 512], F32, tag="ps")
                    for i, g in enumerate(gs):
                        nc.tensor.matmul(
                            ps[:],
                            ysl[:, g, 128 * tt2:128 * (tt2 + 1)],
                            wp_t[:, g, 512 * ec:512 * (ec + 1)],
                            start=(i == 0), stop=(i == len(gs) - 1))
                    dst = osb[:, tt2, 512 * ec:512 * (ec + 1)]
                    if second:
                        nc.vector.tensor_add(dst, dst, ps[:])
                    else:
                        nc.vector.tensor_copy(dst, ps[:])
                if second:
                    row = cq0 + 128 * tt2
                    nc.sync.dma_start(
                        out=out[row:row + 128, :],
                        in_=osb[:, tt2, :])

        def emit_chunks():
          prev = [None]  # (cq0, ysl) awaiting out-projection
          for tq, (cq0, Wc) in enumerate(CHUNKS):
              # ---------------- QKV projections for this chunk ----------------
              xt = xtp.tile([128, 4, 8, 128], FP16, tag="xt")
              splitx = first[0]
              if first[0]:
                  # interleave weight loads with the four x t-slices; the
                  # first Q matmuls need only w[m=0] + x half 0
                  w_dma(0, 1)
                  nc.sync.dma_start(out=xt[:, 0], in_=xT4[:, tq, 0])
                  nc.sync.dma_start(out=xt[:, 1], in_=xT4[:, tq, 1])
                  w_dma(1, 4)
                  nc.sync.dma_start(out=xt[:, 2], in_=xT4[:, tq, 2])
                  nc.sync.dma_start(out=xt[:, 3], in_=xT4[:, tq, 3])
                  w_dma(4, 8)
                  nc.sync.dma_start(out=wv_t[:], in_=wvT[:])
                  nc.sync.dma_start(out=mask_t[:], in_=mask2[:])
                  nc.sync.dma_start(out=sel_t[:], in_=sel[:])
                  nc.sync.dma_start(out=wp_t[:], in_=wpT[:])
                  first[0] = False
              else:
                  nc.sync.dma_start(out=xt[:], in_=xT4[:, tq])

              # chunk-0 single-shot: x lands in slices -> accumulate per
              # 256-t half; otherwise one N=512 group per (m, c) so each
              # weight tile is loaded once
              halves = [(0, 4)] if not splitx else [(0, 2), (2, 4)]
              qsl = qslp.tile([128, NP, 512], FP16, tag="qsl")
              for m in range(8):  # Q pairs then K pairs
                  ps = pp.tile([128, 512], F32, tag="ps")
                  for h0, h1 in halves:
                      for c in range(8):
                          nc.tensor.matmul(ps[:, 128 * h0:128 * h1],
                                           w_all[:, m, c, :],
                                           xt[:, h0:h1, c, :],
                                           start=(c == 0), stop=(c == 7))
                  if m < 4:
                      nc.vector.tensor_copy(qsl[:, m, :], ps[:])
                  else:
                      nc.vector.tensor_copy(kt_t[:, m - 4, cq0:cq0 + Wc],
                                            ps[:])
              for tt in range(4):  # V for the t-tiles of this chunk
                  ps = pp.tile([128, 512], F32, tag="ps")
                  for c in range(8):
                      nc.tensor.matmul(ps[:], xt[:, tt, c, :], wv_t[:, c, :],
                                       start=(c == 0), stop=(c == 7))
                  l = cq0 // 128 + tt
                  psr = ps[:].rearrange("p (hp par d) -> p hp par d",
                                        par=2, d=64)
                  vv4 = vv_t[:, l, :, :].rearrange(
                      "p (hp par) d -> p hp par d", par=2)
                  nc.vector.tensor_copy(vv4[:, :, 0, 0:64], psr[:, :, 0, :])
                  nc.vector.tensor_copy(vv4[:, :, 1, 64:128], psr[:, :, 1, :])

              # previous chunk's output projection: emitted after this
              # chunk's projections so the pp pool ring favors them; runs
              # as PE gap-filler during this chunk's attention.
              if prev[0] is not None:
                  emit_out_proj(*prev[0])
                  prev[0] = None

              # ------- attention + normalize, by head-pair group -------
              W = Wc
              q0 = cq0
              nkb = (q0 + W) // 128   # total k-blocks for this chunk
              kb0 = q0 // 128         # first (partial) diagonal block

              ysl = yslp.tile([128, NP, 512], FP16, tag="ysl")
              osb3 = None
              if tq == 3:
                  osb3 = osp.tile([128, 4, 1024], FP16, tag="osb")
              for g in range(NP):
                  hA, hB = 2 * g, 2 * g + 1
                  yA = ytp.tile([128, 512], F32, tag="ytps")
                  yB = ytp.tile([128, 512], F32, tag="ytps")
                  nb = nkb
                  for l in range(nkb):
                      off = 128 * l - q0 if l >= kb0 else 0
                      st = stp.tile([128, 2, 512], F32, tag="st")
                      nc.tensor.matmul(
                          st[:, 0, off:W],
                          kt_t[0:64, g, 128 * l:128 * (l + 1)],
                          qsl[0:64, g, off:W],
                          start=True, stop=True)
                      nc.tensor.matmul(
                          st[:, 1, off:W],
                          kt_t[64:128, g, 128 * l:128 * (l + 1)],
                          qsl[64:128, g, off:W],
                          start=True, stop=True)
                      e = ep.tile([128, 2, 512], FP16, tag="e")
                      nc.scalar.activation(e[:, :, off:W], st[:, :, off:W],
                                           EXPF, scale=0.125)
                      if l >= kb0:
                          nc.vector.tensor_mul(e[:, :, off:off + 128],
                                               e[:, :, off:off + 128],
                                               mask_t[:])
                      fl = dict(start=(l == 0), stop=(l == nb - 1))
                      # head A: yA rows 0:64 = YT_A, 64:128 = dens_A copies
                      nc.tensor.matmul(yA[:, off:W], vv_t[:, l, hA, :],
                                       e[:, 0, off:W], **fl)
                      # head B: yB rows 0:64 = dens_B copies, 64:128 = YT_B
                      nc.tensor.matmul(yB[:, off:W], vv_t[:, l, hB, :],
                                       e[:, 1, off:W], **fl)

                  # bounce accumulators to SBUF, then realign denominators
                  # onto the YT partitions with two selection matmuls
                  ycp = rpp.tile([128, 2, 512], FP16, tag="ycp")
                  nc.vector.tensor_copy(ycp[:, 0, 0:W], yA[:, 0:W])
                  nc.vector.tensor_copy(ycp[:, 1, 0:W], yB[:, 0:W])
                  # K=64 selections on disjoint row/col groups -> the two
                  # matmuls run concurrently in the array (like the QK pair)
                  den = ytp.tile([128, 512], F32, tag="ytps")
                  nc.tensor.matmul(den[0:64, 0:W], sel_t[64:128, 0, 0:64],
                                   ycp[64:128, 0, 0:W],
                                   start=True, stop=True)
                  nc.tensor.matmul(den[64:128, 0:W], sel_t[0:64, 1, 64:128],
                                   ycp[0:64, 1, 0:W],
                                   start=True, stop=True)
                  rep = rpp.tile([128, 512], F32, tag="rep")
                  nc.vector.reciprocal(rep[:, 0:W], den[:, 0:W])
                  # ysl rows 0:64 = YT_A/dens_A, 64:128 = YT_B/dens_B via
                  # one multiply: [YT_A; YT_B] is ycp cols [0 rows 0:64,
                  # 1 rows 64:128] -- do it as two half multiplies
                  nc.vector.tensor_mul(ysl[0:64, g, 0:W],
                                       ycp[0:64, 0, 0:W], rep[0:64, 0:W])
                  nc.vector.tensor_mul(ysl[64:128, g, 0:W],
                                       ycp[64:128, 1, 0:W],
                                       rep[64:128, 0:W])
                  if tq == 3 and g == 1:
                      # first half of the final projection overlaps the
                      # remaining two groups' attention
                      emit_out_proj_half(cq0, ysl, (0, 1), osb3)

              if tq < 3:
                  prev[0] = (cq0, ysl)
              else:
                  emit_out_proj_half(cq0, ysl, (2, 3), osb3)

        if repeat > 1:
            with tc.For_i(0, repeat, 1):
                emit_chunks()
        else:
            emit_chunks()


def make_core_inputs(x, w_attn, w_proj):
    """Host-side sharding: returns list of 8 input dicts."""
    x = np.asarray(x, dtype=np.float32)
    w_attn = np.asarray(w_attn, dtype=np.float32)
    w_proj = np.asarray(w_proj, dtype=np.float32)
    k = np.arange(128)
    m = (k[None, :] >= k[:, None]).astype(np.float16)
    mask2 = np.ascontiguousarray(np.stack([m, m], axis=1))  # [128, 2, 128]
    sel = np.zeros((128, 2, 128), dtype=np.float16)
    sel[64, 0, 0:64] = 1.0    # out[0:64]   <- rhs[64] (dens_A)
    sel[0, 1, 64:128] = 1.0   # out[64:128] <- rhs[0]  (dens_B)
    in_maps = []
    for core in range(8):
        b, hg = divmod(core, 2)
        cs = 512 * hg
        # x: [c, t] -> [p, tq, ts, c8, t128]
        xb = x[b].T.astype(np.float16)            # [1024, 2048]
        xT4 = np.ascontiguousarray(
            xb.reshape(8, 128, 4, 4, 128).transpose(1, 2, 3, 0, 4))
        # Q/K weights: [1024, 1024] -> [p, m, c, n]
        qk = np.concatenate([w_attn[:, cs:cs + 512],
                             w_attn[:, 1024 + cs:1024 + cs + 512]],
                            axis=1).astype(np.float16)
        wqkvT = np.ascontiguousarray(
            qk.reshape(8, 128, 8, 128).transpose(1, 2, 0, 3))
        # V weights: [1024, 512] -> [p, c, n]
        wv = w_attn[:, 2048 + cs:2048 + cs + 512].astype(np.float16)
        wvT = np.ascontiguousarray(wv.reshape(8, 128, 512).transpose(1, 0, 2))
        # out-proj weights: [512, 1024] -> [p, g, n]
        wp = w_proj[cs:cs + 512, :].astype(np.float16)
        wpT = np.ascontiguousarray(wp.reshape(4, 128, 1024).transpose(1, 0, 2))
        in_maps.append({
            "xT4": xT4,
            "wqkvT": wqkvT,
            "wvT": wvT,
            "wpT": wpT,
            "mask2": mask2,
            "sel": sel,
        })
    return in_maps


_NC_CACHE = {}


def get_nc(repeat=1):
    key = f"nc{repeat}"
    if key not in _NC_CACHE:
        _NC_CACHE[key] = build_nc(repeat=repeat)
    return _NC_CACHE[key]


def kernel(x, w_attn, w_proj):
    from concourse.bass_utils import run_bass_kernel_spmd
    nc = get_nc()
    in_maps = make_core_inputs(x, w_attn, w_proj)
    res = run_bass_kernel_spmd(nc, in_maps, list(range(8)), trace=False)
    parts = [res.results[i]["out"].astype(np.float32) for i in range(8)]
    y = np.stack([parts[2 * b] + parts[2 * b + 1] for b in range(4)], axis=0)
    return y.astype(np.float32)


# revision 28
# speedup vs baseline: 1.5280x; 1.5280x over previous
"""Causal self-attention (4, 2048, 1024), 16 heads, on 8 trn2 NeuronCores.

Sharding: batch (4) x head-group (2 groups of 8 heads) -> 8 cores.
Each core computes, for its batch b and its 8 heads:
  qkv projection -> causal attention -> partial output projection
  partial_out = Y_heads @ w_proj[rows of those heads]
Host sums the two head-group partials per batch. No collectives.

v2 changes vs v1 (296us baseline):
- Host-side tensors pre-arranged in exact SBUF layouts so every load is a
  few large-descriptor DMAs; chunk-0 x arrives in four 128-t slices so
  projections start ~3us into the kernel instead of ~18us.
- Projection matmuls: chunk-0 single-shot accumulates per 256-t half (x
  arrives in slices); all other chunks use one N=512 group per (m, c) so
  each weight tile is loaded once.
- Softmax denominators: the [V|ones] / [ones|V] packing puts 64 copies of
  each head's denominator row in the PV accumulator; two K=64 fp16
  selection matmuls on disjoint row/col groups (concurrent in the array,
  like the QK pair) realign [dens_A; dens_B] onto the same partitions as
  [YT_A; YT_B], replacing the per-group SBUF->SBUF DMA broadcasts that sat
  on the normalize critical path.  One reciprocal per group.
- Output in fp16, staged per chunk in SBUF and stored with one DMA per
  chunk issued from the Pool (gpsimd) queue so input prefetches on the SP
  queue are never stuck behind stores.  Host upcasts and sums partials.
- vv ones-halves initialized by Pool-engine memsets (off the DVE).
- PSUM pools: pp (proj + out-proj + den) 2 banks, st 2x2 banks, yt 2 banks.
"""

import numpy as np

import concourse.bass as bass
import concourse.mybir as mybir
import concourse.tile as tile
from concourse import bacc

F32 = mybir.dt.float32
F32R = mybir.dt.float32r
FP16 = mybir.dt.float16

T = 2048   # sequence length
C = 1024   # embed dim
NP = 4     # head pairs per core (8 heads)
NKT = 16   # k-tiles of 128
EXPF = mybir.ActivationFunctionType.Exp


def r(ap):
    return ap.bitcast(F32R)


def build_nc(repeat=1):
    nc = bacc.Bacc(trn_type="TRN2", target_bir_lowering=False, debug=False,
                   num_devices=8)
    # x, transposed+tiled: [p, tq(4), ts(4), c(8), t(128)]
    xT4 = nc.dram_tensor("xT4", [128, 4, 4, 8, 128], FP16,
                         kind="ExternalInput").ap()
    # Q/K proj weights: [p, m(8: q pairs 0-3, k pairs 4-7), c(8), n(128)]
    wqkvT = nc.dram_tensor("wqkvT", [128, 8, 8, 128], FP16,
                           kind="ExternalInput").ap()
    # V proj weights: [p, c(8), n(512)]
    wvT = nc.dram_tensor("wvT", [128, 8, 512], FP16,
                         kind="ExternalInput").ap()
    # out proj weights: [p, g(4), n(1024)]
    wpT = nc.dram_tensor("wpT", [128, 4, 1024], FP16,
                         kind="ExternalInput").ap()
    # mask2[k, i, q] = 1.0 if q >= k else 0 (same for i=0,1)
    mask2 = nc.dram_tensor("mask2", [128, 2, 128], FP16,
                           kind="ExternalInput").ap()
    # selection matrices for denominator realignment:
    # sel[:, 0]: out[m in 0:64] = rhs[64]   (dens_A from partitions 64:128)
    # sel[:, 1]: out[m in 64:128] = rhs[0]  (dens_B from partitions 0:64)
    sel = nc.dram_tensor("sel", [128, 2, 128], FP16,
                         kind="ExternalInput").ap()
    out = nc.dram_tensor("out", [T, C], FP16, kind="ExternalOutput").ap()

    with tile.TileContext(nc) as tc:
        build_body(tc, xT4, wqkvT, wvT, wpT, mask2, sel, out, repeat=repeat)
    nc.compile()
    return nc


def build_body(tc, xT4, wqkvT, wvT, wpT, mask2, sel, out, repeat=1):
    nc = tc.nc
    import contextlib
    ctx = contextlib.ExitStack()
    with ctx:
        persist = ctx.enter_context(tc.tile_pool(name="persist", bufs=1))
        xtp = ctx.enter_context(tc.tile_pool(name="xt_p", bufs=2))
        qslp = ctx.enter_context(tc.tile_pool(name="qsl_p", bufs=2))
        yslp = ctx.enter_context(tc.tile_pool(name="ysl_p", bufs=3))
        ep = ctx.enter_context(tc.tile_pool(name="e_p", bufs=12))
        rpp = ctx.enter_context(tc.tile_pool(name="rep_p", bufs=2))
        osp = ctx.enter_context(tc.tile_pool(name="osb_p", bufs=2))
        pp = ctx.enter_context(tc.tile_pool(name="pp_ps", bufs=2,
                                            space="PSUM"))
        stp = ctx.enter_context(tc.tile_pool(name="st_ps", bufs=2,
                                             space="PSUM"))
        ytp = ctx.enter_context(tc.tile_pool(name="yt_ps", bufs=2,
                                             space="PSUM"))

        kt_t = persist.tile([128, NP, T], FP16)      # KT pairs (d=128, t)
        # V+ones: per (ktile, head) block of 128 cols:
        # even heads [V_h | ones], odd heads [ones | V_h]
        vv_t = persist.tile([128, NKT, 8, 128], FP16)
        mask_t = persist.tile([128, 2, 128], FP16)
        sel_t = persist.tile([128, 2, 128], FP16)
        wv_t = persist.tile([128, 8, 512], FP16)     # V proj weights
        w_all = persist.tile([128, 8, 8, 128], FP16)  # QT/KT proj weights
        wp_t = persist.tile([128, NP, C], FP16)      # out proj weights

        # ---- one-time loads, interleaved with chunk-0 x slices on the SP
        # queue so the first projection matmuls start as early as possible.
        def w_dma(m0, m1):
            nc.sync.dma_start(out=w_all[:, m0:m1, :, :],
                              in_=wqkvT[:, m0:m1, :, :])

        # vv ones-halves (never overwritten by V copies) on the Pool engine,
        # split so the first key-tiles are ready early.
        vv5 = vv_t[:, :, :, :].rearrange("p l (hp par) d -> p l hp par d",
                                         par=2)
        for lh in range(2):
            sl = slice(8 * lh, 8 * lh + 8)
            nc.gpsimd.memset(vv5[:, sl, :, 0, 64:128], 1.0)
            nc.gpsimd.memset(vv5[:, sl, :, 1, 0:64], 1.0)

        CHUNKS = [(0, 512), (512, 512), (1024, 512), (1536, 512)]
        first = [True]
        if repeat > 1:
            # steady-state (slope) build: persist loads once, outside the
            # hardware loop, so iterations carry no reload overhead
            first[0] = False
            w_dma(0, 8)
            nc.sync.dma_start(out=wv_t[:], in_=wvT[:])
            nc.sync.dma_start(out=mask_t[:], in_=mask2[:])
            nc.sync.dma_start(out=sel_t[:], in_=sel[:])
            nc.sync.dma_start(out=wp_t[:], in_=wpT[:])

        def emit_out_proj(cq0, ysl):
            # output projection + one fp16 store per chunk (Pool queue)
            osb = osp.tile([128, 4, 1024], FP16, tag="osb")
            for tt2 in range(4):
                for ec in range(2):
                    ps = pp.tile([128, 512], F32, tag="ps")
                    for g in range(NP):
                        nc.tensor.matmul(
                            ps[:],
                            ysl[:, g, 128 * tt2:128 * (tt2 + 1)],
                            wp_t[:, g, 512 * ec:512 * (ec + 1)],
                            start=(g == 0), stop=(g == 3))
                    nc.vector.tensor_copy(
                        osb[:, tt2, 512 * ec:512 * (ec + 1)], ps[:])
            nc.gpsimd.dma_start(
                out=out[cq0:cq0 + 512, :].rearrange(
                    "(tt p) n -> p tt n", p=128),
                in_=osb[:])

        def emit_out_proj_half(cq0, ysl, gs, osb):
            # partial final-chunk projection over head-pair groups gs; the
            # second pass adds onto the first (adds split DVE/Pool) and
            # stores per 128-row piece
            second = gs[0] != 0
            for tt2 in range(4):
                for ec in range(2):
                    ps = pp.tile([128, 512], F32, tag="ps")
                    for i, g in enumerate(gs):
                        nc.tensor.matmul(
                            ps[:],
                            ysl[:, g, 128 * tt2:128 * (tt2 + 1)],
                            wp_t[:, g, 512 * ec:512 * (ec + 1)],
                            start=(i == 0), stop=(i == len(gs) - 1))
                    dst = osb[:, tt2, 512 * ec:512 * (ec + 1)]
                    if second:
                        nc.vector.tensor_add(dst, dst, ps[:])
                    else:
                        nc.vector.tensor_copy(dst, ps[:])
                if second:
                    row = cq0 + 128 * tt2
                    nc.sync.dma_start(
                        out=out[row:row + 128, :],
                        in_=osb[:, tt2, :])

        def emit_chunks():
          prev = []  # [(cq0, ysl)] awaiting out-projection (2-deep)
          for tq, (cq0, Wc) in enumerate(CHUNKS):
              # ---------------- QKV projections for this chunk ----------------
              xt = xtp.tile([128, 4, 8, 128], FP16, tag="xt")
              splitx = first[0]
              if first[0]:
                  # interleave weight loads with the four x t-slices; the
                  # first Q matmuls need only w[m=0] + x half 0
                  w_dma(0, 1)
                  nc.sync.dma_start(out=xt[:, 0], in_=xT4[:, tq, 0])
                  nc.sync.dma_start(out=xt[:, 1], in_=xT4[:, tq, 1])
                  w_dma(1, 4)
                  nc.sync.dma_start(out=xt[:, 2], in_=xT4[:, tq, 2])
                  nc.sync.dma_start(out=xt[:, 3], in_=xT4[:, tq, 3])
                  w_dma(4, 8)
                  nc.sync.dma_start(out=wv_t[:], in_=wvT[:])
                  nc.sync.dma_start(out=mask_t[:], in_=mask2[:])
                  nc.sync.dma_start(out=sel_t[:], in_=sel[:])
                  nc.sync.dma_start(out=wp_t[:], in_=wpT[:])
                  first[0] = False
              else:
                  nc.sync.dma_start(out=xt[:], in_=xT4[:, tq])

              # chunk-0 single-shot: x lands in slices -> accumulate per
              # 256-t half; otherwise one N=512 group per (m, c) so each
              # weight tile is loaded once
              halves = [(0, 4)] if not splitx else [(0, 2), (2, 4)]
              qsl = qslp.tile([128, NP, 512], FP16, tag="qsl")
              for m in range(8):  # Q pairs then K pairs
                  ps = pp.tile([128, 512], F32, tag="ps")
                  for h0, h1 in halves:
                      for c in range(8):
                          nc.tensor.matmul(ps[:, 128 * h0:128 * h1],
                                           w_all[:, m, c, :],
                                           xt[:, h0:h1, c, :],
                                           start=(c == 0), stop=(c == 7))
                  if m < 4:
                      nc.vector.tensor_copy(qsl[:, m, :], ps[:])
                  else:
                      nc.vector.tensor_copy(kt_t[:, m - 4, cq0:cq0 + Wc],
                                            ps[:])
              for tt in range(4):  # V for the t-tiles of this chunk
                  ps = pp.tile([128, 512], F32, tag="ps")
                  for c in range(8):
                      nc.tensor.matmul(ps[:], xt[:, tt, c, :], wv_t[:, c, :],
                                       start=(c == 0), stop=(c == 7))
                  l = cq0 // 128 + tt
                  psr = ps[:].rearrange("p (hp par d) -> p hp par d",
                                        par=2, d=64)
                  vv4 = vv_t[:, l, :, :].rearrange(
                      "p (hp par) d -> p hp par d", par=2)
                  nc.vector.tensor_copy(vv4[:, :, 0, 0:64], psr[:, :, 0, :])
                  nc.vector.tensor_copy(vv4[:, :, 1, 64:128], psr[:, :, 1, :])

              # deferred output projections: emitted after this chunk's
              # projections so the pp pool ring favors them; they run as
              # PE gap-filler during this chunk's ACT-bound attention.
              # Two chunks deep so the later (ACT-bound) windows get more
              # filler and the PE-heavy middle windows get less.
              if tq == 2 or tq == 3:
                  emit_out_proj(*prev.pop(0))

              # ------- attention + normalize, by head-pair group -------
              W = Wc
              q0 = cq0
              nkb = (q0 + W) // 128   # total k-blocks for this chunk
              kb0 = q0 // 128         # first (partial) diagonal block

              ysl = yslp.tile([128, NP, 512], FP16, tag="ysl")
              osb3 = None
              if tq == 3:
                  osb3 = osp.tile([128, 4, 1024], FP16, tag="osb")
              for g in range(NP):
                  hA, hB = 2 * g, 2 * g + 1
                  yA = ytp.tile([128, 512], F32, tag="ytps")
                  yB = ytp.tile([128, 512], F32, tag="ytps")
                  nb = nkb
                  for l in range(nkb):
                      off = 128 * l - q0 if l >= kb0 else 0
                      st = stp.tile([128, 2, 512], F32, tag="st")
                      nc.tensor.matmul(
                          st[:, 0, off:W],
                          kt_t[0:64, g, 128 * l:128 * (l + 1)],
                          qsl[0:64, g, off:W],
                          start=True, stop=True)
                      nc.tensor.matmul(
                          st[:, 1, off:W],
                          kt_t[64:128, g, 128 * l:128 * (l + 1)],
                          qsl[64:128, g, off:W],
                          start=True, stop=True)
                      e = ep.tile([128, 2, 512], FP16, tag="e")
                      nc.scalar.activation(e[:, :, off:W], st[:, :, off:W],
                                           EXPF, scale=0.125)
                      if l >= kb0:
                          nc.vector.tensor_mul(e[:, :, off:off + 128],
                                               e[:, :, off:off + 128],
                                               mask_t[:])
                      fl = dict(start=(l == 0), stop=(l == nb - 1))
                      # head A: yA rows 0:64 = YT_A, 64:128 = dens_A copies
                      nc.tensor.matmul(yA[:, off:W], vv_t[:, l, hA, :],
                                       e[:, 0, off:W], **fl)
                      # head B: yB rows 0:64 = dens_B copies, 64:128 = YT_B
                      nc.tensor.matmul(yB[:, off:W], vv_t[:, l, hB, :],
                                       e[:, 1, off:W], **fl)

                  # bounce accumulators to SBUF, then realign denominators
                  # onto the YT partitions with two selection matmuls
                  ycp = rpp.tile([128, 2, 512], FP16, tag="ycp")
                  nc.vector.tensor_copy(ycp[:, 0, 0:W], yA[:, 0:W])
                  nc.vector.tensor_copy(ycp[:, 1, 0:W], yB[:, 0:W])
                  # K=64 selections on disjoint row/col groups -> the two
                  # matmuls run concurrently in the array (like the QK pair)
                  den = ytp.tile([128, 512], F32, tag="ytps")
                  nc.tensor.matmul(den[0:64, 0:W], sel_t[64:128, 0, 0:64],
                                   ycp[64:128, 0, 0:W],
                                   start=True, stop=True)
                  nc.tensor.matmul(den[64:128, 0:W], sel_t[0:64, 1, 64:128],
                                   ycp[0:64, 1, 0:W],
                                   start=True, stop=True)
                  rep = rpp.tile([128, 512], F32, tag="rep")
                  nc.vector.reciprocal(rep[:, 0:W], den[:, 0:W])
                  # ysl rows 0:64 = YT_A/dens_A, 64:128 = YT_B/dens_B via
                  # one multiply: [YT_A; YT_B] is ycp cols [0 rows 0:64,
                  # 1 rows 64:128] -- do it as two half multiplies
                  nc.vector.tensor_mul(ysl[0:64, g, 0:W],
                                       ycp[0:64, 0, 0:W], rep[0:64, 0:W])
                  nc.vector.tensor_mul(ysl[64:128, g, 0:W],
                                       ycp[64:128, 1, 0:W],
                                       rep[64:128, 0:W])
                  if tq == 3 and g == 0:
                      # chunk-2's projection fills the last three groups'
                      # ACT-bound attention
                      emit_out_proj(*prev.pop(0))
                  if tq == 3 and g == 1:
                      # first half of the final projection overlaps the
                      # remaining two groups' attention
                      emit_out_proj_half(cq0, ysl, (0, 1), osb3)

              if tq < 3:
                  prev.append((cq0, ysl))
              else:
                  emit_out_proj_half(cq0, ysl, (2, 3), osb3)

        if repeat > 1:
            with tc.For_i(0, repeat, 1):
                emit_chunks()
        else:
            emit_chunks()


def make_core_inputs(x, w_attn, w_proj):
    """Host-side sharding: returns list of 8 input dicts."""
    x = np.asarray(x, dtype=np.float32)
    w_attn = np.asarray(w_attn, dtype=np.float32)
    w_proj = np.asarray(w_proj, dtype=np.float32)
    k = np.arange(128)
    m = (k[None, :] >= k[:, None]).astype(np.float16)
    mask2 = np.ascontiguousarray(np.stack([m, m], axis=1))  # [128, 2, 128]
    sel = np.zeros((128, 2, 128), dtype=np.float16)
    sel[64, 0, 0:64] = 1.0    # out[0:64]   <- rhs[64] (dens_A)
    sel[0, 1, 64:128] = 1.0   # out[64:128] <- rhs[0]  (dens_B)
    in_maps = []
    for core in range(8):
        b, hg = divmod(core, 2)
        cs = 512 * hg
        # x: [c, t] -> [p, tq, ts, c8, t128]
        xb = x[b].T.astype(np.float16)            # [1024, 2048]
        xT4 = np.ascontiguousarray(
            xb.reshape(8, 128, 4, 4, 128).transpose(1, 2, 3, 0, 4))
        # Q/K weights: [1024, 1024] -> [p, m, c, n]
        qk = np.concatenate([w_attn[:, cs:cs + 512],
                             w_attn[:, 1024 + cs:1024 + cs + 512]],
                            axis=1).astype(np.float16)
        wqkvT = np.ascontiguousarray(
            qk.reshape(8, 128, 8, 128).transpose(1, 2, 0, 3))
        # V weights: [1024, 512] -> [p, c, n]
        wv = w_attn[:, 2048 + cs:2048 + cs + 512].astype(np.float16)
        wvT = np.ascontiguousarray(wv.reshape(8, 128, 512).transpose(1, 0, 2))
        # out-proj weights: [512, 1024] -> [p, g, n]
        wp = w_proj[cs:cs + 512, :].astype(np.float16)
        wpT = np.ascontiguousarray(wp.reshape(4, 128, 1024).transpose(1, 0, 2))
        in_maps.append({
            "xT4": xT4,
            "wqkvT": wqkvT,
            "wvT": wvT,
            "wpT": wpT,
            "mask2": mask2,
            "sel": sel,
        })
    return in_maps


_NC_CACHE = {}


def get_nc(repeat=1):
    key = f"nc{repeat}"
    if key not in _NC_CACHE:
        _NC_CACHE[key] = build_nc(repeat=repeat)
    return _NC_CACHE[key]


def kernel(x, w_attn, w_proj):
    from concourse.bass_utils import run_bass_kernel_spmd
    nc = get_nc()
    in_maps = make_core_inputs(x, w_attn, w_proj)
    res = run_bass_kernel_spmd(nc, in_maps, list(range(8)), trace=False)
    parts = [res.results[i]["out"].astype(np.float32) for i in range(8)]
    y = np.stack([parts[2 * b] + parts[2 * b + 1] for b in range(4)], axis=0)
    return y.astype(np.float32)


# revision 31
# speedup vs baseline: 2.0984x; 1.3733x over previous
"""Causal self-attention (4, 2048, 1024), 16 heads, on 8 trn2 NeuronCores.

Sharding: batch (4) x head-group (2 groups of 8 heads) -> 8 cores.
Each core computes, for its batch b and its 8 heads:
  qkv projection -> causal attention -> partial output projection
  partial_out = Y_heads @ w_proj[rows of those heads]
Host sums the two head-group partials per batch. No collectives.

v2 changes vs v1 (296us baseline):
- Host-side tensors pre-arranged in exact SBUF layouts so every load is a
  few large-descriptor DMAs; chunk-0 x arrives in four 128-t slices so
  projections start ~3us into the kernel instead of ~18us.
- Projection matmuls: chunk-0 single-shot accumulates per 256-t half (x
  arrives in slices); all other chunks use one N=512 group per (m, c) so
  each weight tile is loaded once.
- Softmax denominators: the [V|ones] / [ones|V] packing puts 64 copies of
  each head's denominator row in the PV accumulator; two K=64 fp16
  selection matmuls on disjoint row/col groups (concurrent in the array,
  like the QK pair) realign [dens_A; dens_B] onto the same partitions as
  [YT_A; YT_B], replacing the per-group SBUF->SBUF DMA broadcasts that sat
  on the normalize critical path.  One reciprocal per group.
- Output in fp16, staged per chunk in SBUF and stored with one DMA per
  chunk issued from the Pool (gpsimd) queue so input prefetches on the SP
  queue are never stuck behind stores.  Host upcasts and sums partials.
- vv ones-halves initialized by Pool-engine memsets (off the DVE).
- PSUM pools: pp (proj + out-proj + den) 2 banks, st 2x2 banks, yt 2 banks.
"""

import numpy as np

import concourse.bass as bass
import concourse.mybir as mybir
import concourse.tile as tile
from concourse import bacc

F32 = mybir.dt.float32
F32R = mybir.dt.float32r
FP16 = mybir.dt.float16

T = 2048   # sequence length
C = 1024   # embed dim
NP = 4     # head pairs per core (8 heads)
NKT = 16   # k-tiles of 128
EXPF = mybir.ActivationFunctionType.Exp


def r(ap):
    return ap.bitcast(F32R)


def build_nc(repeat=1):
    nc = bacc.Bacc(trn_type="TRN2", target_bir_lowering=False, debug=False,
                   num_devices=8)
    # x, transposed+tiled: [p, tq(4), ts(4), c(8), t(128)]
    xT4 = nc.dram_tensor("xT4", [128, 4, 4, 8, 128], FP16,
                         kind="ExternalInput").ap()
    # Q/K proj weights: [p, m(8: q pairs 0-3, k pairs 4-7), c(8), n(128)]
    wqkvT = nc.dram_tensor("wqkvT", [128, 8, 8, 128], FP16,
                           kind="ExternalInput").ap()
    # V proj weights: [p, c(8), n(512)]
    wvT = nc.dram_tensor("wvT", [128, 8, 512], FP16,
                         kind="ExternalInput").ap()
    # out proj weights: [p, g(4), n(1024)]
    wpT = nc.dram_tensor("wpT", [128, 4, 1024], FP16,
                         kind="ExternalInput").ap()
    # mask2[k, i, q] = 1.0 if q >= k else 0 (same for i=0,1)
    mask2 = nc.dram_tensor("mask2", [128, 2, 128], FP16,
                           kind="ExternalInput").ap()
    # selection matrices for denominator realignment:
    # sel[:, 0]: out[m in 0:64] = rhs[64]   (dens_A from partitions 64:128)
    # sel[:, 1]: out[m in 64:128] = rhs[0]  (dens_B from partitions 0:64)
    sel = nc.dram_tensor("sel", [128, 2, 128], FP16,
                         kind="ExternalInput").ap()
    out = nc.dram_tensor("out", [T, C], FP16, kind="ExternalOutput").ap()

    with tile.TileContext(nc) as tc:
        build_body(tc, xT4, wqkvT, wvT, wpT, mask2, sel, out, repeat=repeat)
    nc.compile()
    return nc


def build_body(tc, xT4, wqkvT, wvT, wpT, mask2, sel, out, repeat=1):
    nc = tc.nc
    import contextlib
    ctx = contextlib.ExitStack()
    with ctx:
        persist = ctx.enter_context(tc.tile_pool(name="persist", bufs=1))
        xtp = ctx.enter_context(tc.tile_pool(name="xt_p", bufs=2))
        qslp = ctx.enter_context(tc.tile_pool(name="qsl_p", bufs=2))
        yslp = ctx.enter_context(tc.tile_pool(name="ysl_p", bufs=3))
        ep = ctx.enter_context(tc.tile_pool(name="e_p", bufs=12))
        rpp = ctx.enter_context(tc.tile_pool(name="rep_p", bufs=2))
        osp = ctx.enter_context(tc.tile_pool(name="osb_p", bufs=2))
        pp = ctx.enter_context(tc.tile_pool(name="pp_ps", bufs=2,
                                            space="PSUM"))
        stp = ctx.enter_context(tc.tile_pool(name="st_ps", bufs=2,
                                             space="PSUM"))
        ytp = ctx.enter_context(tc.tile_pool(name="yt_ps", bufs=2,
                                             space="PSUM"))

        kt_t = persist.tile([128, NP, T], FP16)      # KT pairs (d=128, t)
        # V+ones: per (ktile, head) block of 128 cols:
        # even heads [V_h | ones], odd heads [ones | V_h]
        vv_t = persist.tile([128, NKT, 8, 128], FP16)
        mask_t = persist.tile([128, 2, 128], FP16)
        sel_t = persist.tile([128, 2, 128], FP16)
        wv_t = persist.tile([128, 8, 512], FP16)     # V proj weights
        w_all = persist.tile([128, 8, 8, 128], FP16)  # QT/KT proj weights
        wp_t = persist.tile([128, NP, C], FP16)      # out proj weights

        # ---- one-time loads, interleaved with chunk-0 x slices on the SP
        # queue so the first projection matmuls start as early as possible.
        def w_dma(m0, m1):
            nc.sync.dma_start(out=w_all[:, m0:m1, :, :],
                              in_=wqkvT[:, m0:m1, :, :])

        # vv ones-halves (never overwritten by V copies) on the Pool engine,
        # split so the first key-tiles are ready early.
        vv5 = vv_t[:, :, :, :].rearrange("p l (hp par) d -> p l hp par d",
                                         par=2)
        for lh in range(2):
            sl = slice(8 * lh, 8 * lh + 8)
            nc.gpsimd.memset(vv5[:, sl, :, 0, 64:128], 1.0)
            nc.gpsimd.memset(vv5[:, sl, :, 1, 0:64], 1.0)

        CHUNKS = [(0, 512), (512, 512), (1024, 512), (1536, 512)]
        first = [True]
        if repeat > 1:
            # steady-state (slope) build: persist loads once, outside the
            # hardware loop, so iterations carry no reload overhead
            first[0] = False
            w_dma(0, 8)
            nc.sync.dma_start(out=wv_t[:], in_=wvT[:])
            nc.sync.dma_start(out=mask_t[:], in_=mask2[:])
            nc.sync.dma_start(out=sel_t[:], in_=sel[:])
            nc.sync.dma_start(out=wp_t[:], in_=wpT[:])

        def emit_out_proj(cq0, ysl):
            # output projection + one fp16 store per chunk (Pool queue)
            osb = osp.tile([128, 4, 1024], FP16, tag="osb")
            for tt2 in range(4):
                for ec in range(2):
                    ps = pp.tile([128, 512], F32, tag="ps")
                    for g in range(NP):
                        nc.tensor.matmul(
                            ps[:],
                            ysl[:, g, 128 * tt2:128 * (tt2 + 1)],
                            wp_t[:, g, 512 * ec:512 * (ec + 1)],
                            start=(g == 0), stop=(g == 3))
                    nc.vector.tensor_copy(
                        osb[:, tt2, 512 * ec:512 * (ec + 1)], ps[:])
            nc.gpsimd.dma_start(
                out=out[cq0:cq0 + 512, :].rearrange(
                    "(tt p) n -> p tt n", p=128),
                in_=osb[:])

        def emit_out_proj_half(cq0, ysl, gs, osb):
            # partial final-chunk projection over head-pair groups gs;
            # later passes add onto the first, the last stores per
            # 128-row piece
            second = gs[0] != 0
            store = gs[-1] == 3
            for tt2 in range(4):
                for ec in range(2):
                    ps = pp.tile([128, 512], F32, tag="ps")
                    for i, g in enumerate(gs):
                        nc.tensor.matmul(
                            ps[:],
                            ysl[:, g, 128 * tt2:128 * (tt2 + 1)],
                            wp_t[:, g, 512 * ec:512 * (ec + 1)],
                            start=(i == 0), stop=(i == len(gs) - 1))
                    dst = osb[:, tt2, 512 * ec:512 * (ec + 1)]
                    if second:
                        nc.vector.tensor_add(dst, dst, ps[:])
                    else:
                        nc.vector.tensor_copy(dst, ps[:])
                if store:
                    row = cq0 + 128 * tt2
                    nc.sync.dma_start(
                        out=out[row:row + 128, :],
                        in_=osb[:, tt2, :])

        def emit_chunks():
          prev = []  # [(cq0, ysl)] awaiting out-projection (2-deep)
          for tq, (cq0, Wc) in enumerate(CHUNKS):
              # ---------------- QKV projections for this chunk ----------------
              xt = xtp.tile([128, 4, 8, 128], FP16, tag="xt")
              splitx = first[0]
              if first[0]:
                  # interleave weight loads with the four x t-slices; the
                  # first Q matmuls need only w[m=0] + x half 0
                  w_dma(0, 1)
                  nc.sync.dma_start(out=xt[:, 0], in_=xT4[:, tq, 0])
                  nc.sync.dma_start(out=xt[:, 1], in_=xT4[:, tq, 1])
                  w_dma(1, 4)
                  nc.sync.dma_start(out=xt[:, 2], in_=xT4[:, tq, 2])
                  nc.sync.dma_start(out=xt[:, 3], in_=xT4[:, tq, 3])
                  w_dma(4, 8)
                  nc.sync.dma_start(out=wv_t[:], in_=wvT[:])
                  nc.sync.dma_start(out=mask_t[:], in_=mask2[:])
                  nc.sync.dma_start(out=sel_t[:], in_=sel[:])
                  nc.sync.dma_start(out=wp_t[:], in_=wpT[:])
                  first[0] = False
              else:
                  nc.sync.dma_start(out=xt[:], in_=xT4[:, tq])

              # chunk-0 single-shot: x lands in slices -> accumulate per
              # 256-t half; otherwise one N=512 group per (m, c) so each
              # weight tile is loaded once
              halves = [(0, 4)] if not splitx else [(0, 2), (2, 4)]
              qsl = qslp.tile([128, NP, 512], FP16, tag="qsl")
              for m in range(8):  # Q pairs then K pairs
                  ps = pp.tile([128, 512], F32, tag="ps")
                  for h0, h1 in halves:
                      for c in range(8):
                          nc.tensor.matmul(ps[:, 128 * h0:128 * h1],
                                           w_all[:, m, c, :],
                                           xt[:, h0:h1, c, :],
                                           start=(c == 0), stop=(c == 7))
                  if m < 4:
                      nc.vector.tensor_copy(qsl[:, m, :], ps[:])
                  else:
                      nc.vector.tensor_copy(kt_t[:, m - 4, cq0:cq0 + Wc],
                                            ps[:])
              for tt in range(4):  # V for the t-tiles of this chunk
                  ps = pp.tile([128, 512], F32, tag="ps")
                  for c in range(8):
                      nc.tensor.matmul(ps[:], xt[:, tt, c, :], wv_t[:, c, :],
                                       start=(c == 0), stop=(c == 7))
                  l = cq0 // 128 + tt
                  psr = ps[:].rearrange("p (hp par d) -> p hp par d",
                                        par=2, d=64)
                  vv4 = vv_t[:, l, :, :].rearrange(
                      "p (hp par) d -> p hp par d", par=2)
                  nc.vector.tensor_copy(vv4[:, :, 0, 0:64], psr[:, :, 0, :])
                  nc.vector.tensor_copy(vv4[:, :, 1, 64:128], psr[:, :, 1, :])

              # deferred output projections: emitted after this chunk's
              # projections so the pp pool ring favors them; they run as
              # PE gap-filler during this chunk's ACT-bound attention.
              # Two chunks deep so the later (ACT-bound) windows get more
              # filler and the PE-heavy middle windows get less.
              if tq == 2 or tq == 3:
                  emit_out_proj(*prev.pop(0))

              # ------- attention + normalize, by head-pair group -------
              W = Wc
              q0 = cq0
              nkb = (q0 + W) // 128   # total k-blocks for this chunk
              kb0 = q0 // 128         # first (partial) diagonal block

              ysl = yslp.tile([128, NP, 512], FP16, tag="ysl")
              osb3 = None
              if tq == 3:
                  osb3 = osp.tile([128, 4, 1024], FP16, tag="osb")
              for g in range(NP):
                  hA, hB = 2 * g, 2 * g + 1
                  yA = ytp.tile([128, 512], F32, tag="ytps")
                  yB = ytp.tile([128, 512], F32, tag="ytps")
                  nb = nkb
                  for l in range(nkb):
                      off = 128 * l - q0 if l >= kb0 else 0
                      st = stp.tile([128, 2, 512], F32, tag="st")
                      nc.tensor.matmul(
                          st[:, 0, off:W],
                          kt_t[0:64, g, 128 * l:128 * (l + 1)],
                          qsl[0:64, g, off:W],
                          start=True, stop=True)
                      nc.tensor.matmul(
                          st[:, 1, off:W],
                          kt_t[64:128, g, 128 * l:128 * (l + 1)],
                          qsl[64:128, g, off:W],
                          start=True, stop=True)
                      e = ep.tile([128, 2, 512], FP16, tag="e")
                      nc.scalar.activation(e[:, :, off:W], st[:, :, off:W],
                                           EXPF, scale=0.125)
                      if l >= kb0:
                          nc.vector.tensor_mul(e[:, :, off:off + 128],
                                               e[:, :, off:off + 128],
                                               mask_t[:])
                      fl = dict(start=(l == 0), stop=(l == nb - 1))
                      # head A: yA rows 0:64 = YT_A, 64:128 = dens_A copies
                      nc.tensor.matmul(yA[:, off:W], vv_t[:, l, hA, :],
                                       e[:, 0, off:W], **fl)
                      # head B: yB rows 0:64 = dens_B copies, 64:128 = YT_B
                      nc.tensor.matmul(yB[:, off:W], vv_t[:, l, hB, :],
                                       e[:, 1, off:W], **fl)

                  # bounce accumulators to SBUF, then realign denominators
                  # onto the YT partitions with two selection matmuls
                  ycp = rpp.tile([128, 2, 512], FP16, tag="ycp")
                  nc.vector.tensor_copy(ycp[:, 0, 0:W], yA[:, 0:W])
                  nc.vector.tensor_copy(ycp[:, 1, 0:W], yB[:, 0:W])
                  # K=64 selections on disjoint row/col groups -> the two
                  # matmuls run concurrently in the array (like the QK pair)
                  den = ytp.tile([128, 512], F32, tag="ytps")
                  nc.tensor.matmul(den[0:64, 0:W], sel_t[64:128, 0, 0:64],
                                   ycp[64:128, 0, 0:W],
                                   start=True, stop=True)
                  nc.tensor.matmul(den[64:128, 0:W], sel_t[0:64, 1, 64:128],
                                   ycp[0:64, 1, 0:W],
                                   start=True, stop=True)
                  rep = rpp.tile([128, 512], F32, tag="rep")
                  nc.vector.reciprocal(rep[:, 0:W], den[:, 0:W])
                  # ysl rows 0:64 = YT_A/dens_A, 64:128 = YT_B/dens_B via
                  # one multiply: [YT_A; YT_B] is ycp cols [0 rows 0:64,
                  # 1 rows 64:128] -- do it as two half multiplies
                  nc.vector.tensor_mul(ysl[0:64, g, 0:W],
                                       ycp[0:64, 0, 0:W], rep[0:64, 0:W])
                  nc.vector.tensor_mul(ysl[64:128, g, 0:W],
                                       ycp[64:128, 1, 0:W],
                                       rep[64:128, 0:W])
                  if tq == 3 and g == 0:
                      # chunk-2's projection fills the last three groups'
                      # ACT-bound attention
                      emit_out_proj(*prev.pop(0))
                  if tq == 3 and g == 1:
                      # first half of the final projection overlaps the
                      # remaining two groups' attention
                      emit_out_proj_half(cq0, ysl, (0, 1), osb3)

              if tq < 3:
                  prev.append((cq0, ysl))
              else:
                  emit_out_proj_half(cq0, ysl, (2, 3), osb3)

        if repeat > 1:
            with tc.For_i(0, repeat, 1):
                emit_chunks()
        else:
            emit_chunks()


def make_core_inputs(x, w_attn, w_proj):
    """Host-side sharding: returns list of 8 input dicts."""
    x = np.asarray(x, dtype=np.float32)
    w_attn = np.asarray(w_attn, dtype=np.float32)
    w_proj = np.asarray(w_proj, dtype=np.float32)
    k = np.arange(128)
    m = (k[None, :] >= k[:, None]).astype(np.float16)
    mask2 = np.ascontiguousarray(np.stack([m, m], axis=1))  # [128, 2, 128]
    sel = np.zeros((128, 2, 128), dtype=np.float16)
    sel[64, 0, 0:64] = 1.0    # out[0:64]   <- rhs[64] (dens_A)
    sel[0, 1, 64:128] = 1.0   # out[64:128] <- rhs[0]  (dens_B)
    in_maps = []
    for core in range(8):
        b, hg = divmod(core, 2)
        cs = 512 * hg
        # x: [c, t] -> [p, tq, ts, c8, t128]
        xb = x[b].T.astype(np.float16)            # [1024, 2048]
        xT4 = np.ascontiguousarray(
            xb.reshape(8, 128, 4, 4, 128).transpose(1, 2, 3, 0, 4))
        # Q/K weights: [1024, 1024] -> [p, m, c, n]
        qk = np.concatenate([w_attn[:, cs:cs + 512],
                             w_attn[:, 1024 + cs:1024 + cs + 512]],
                            axis=1).astype(np.float16)
        wqkvT = np.ascontiguousarray(
            qk.reshape(8, 128, 8, 128).transpose(1, 2, 0, 3))
        # V weights: [1024, 512] -> [p, c, n]
        wv = w_attn[:, 2048 + cs:2048 + cs + 512].astype(np.float16)
        wvT = np.ascontiguousarray(wv.reshape(8, 128, 512).transpose(1, 0, 2))
        # out-proj weights: [512, 1024] -> [p, g, n]
        wp = w_proj[cs:cs + 512, :].astype(np.float16)
        wpT = np.ascontiguousarray(wp.reshape(4, 128, 1024).transpose(1, 0, 2))
        in_maps.append({
            "xT4": xT4,
            "wqkvT": wqkvT,
            "wvT": wvT,
            "wpT": wpT,
            "mask2": mask2,
            "sel": sel,
        })
    return in_maps


_NC_CACHE = {}


def get_nc(repeat=1):
    key = f"nc{repeat}"
    if key not in _NC_CACHE:
        _NC_CACHE[key] = build_nc(repeat=repeat)
    return _NC_CACHE[key]


def kernel(x, w_attn, w_proj):
    from concourse.bass_utils import run_bass_kernel_spmd
    nc = get_nc()
    in_maps = make_core_inputs(x, w_attn, w_proj)
    res = run_bass_kernel_spmd(nc, in_maps, list(range(8)), trace=False)
    parts = [res.results[i]["out"].astype(np.float32) for i in range(8)]
    y = np.stack([parts[2 * b] + parts[2 * b + 1] for b in range(4)], axis=0)
    return y.astype(np.float32)
